# revision 44
# baseline (speedup 1.0000x reference)
"""Causal MHA (B=1, T=4096, D=768, H=12) on 8 TRN2 NeuronCores.

Strategy
--------
- Sequence-parallel over T with row-interleaved q-assignment so every core
  runs the *same* program on identically-shaped causal work:
  core c owns q rows {16*(c+8*t)+u}, i.e. 16-row miniblocks strided by 8.
- K/V projection is sharded: each core projects only its own 512-key
  chunk, stages K^T+V' to DRAM, and three pipelined AllGathers (split by
  head-pair group) broadcast them. Gather group 0 (head pairs 0-1)
  launches after ~1/3 of the own-chunk projection, so attention pair 0
  starts while groups 1-2 are still on the wire. Q projection fills the
  remaining gather latency.
- All matmul inputs in bfloat16 (host-side cast): full PE rate at every
  moving-dim size (fp32r drops to 1/4 rate below 256), half the DMA
  bytes, and double DVE throughput on copies/masking. PSUM stays fp32.
- K^T and V' (with the ones-column that folds the softmax denominator
  into the PV matmul) are fully SBUF-resident in bf16.
- Score batches use 16-row-granular causal q-windows (window start
  16*kbs[0]); the per-kb masks encode the boundary with r = kb - kbs[0].
  exp() reads only valid score columns (strided APs) -- the attention
  phase is ACT(exp)-throughput-bound, so every wasted exp column counts.
- Softmax denominator: 1/den = exp(-ln(den)) on the (otherwise idle at
  pair tails) ACT engine, broadcast across the 64 head dims with a
  1-partition PE outer product.
- PSUM: 2 score bufs (2 banks each) + 3 ctx bufs + 1 broadcast buf
  = 8 banks; 3 ctx bufs let the next head pair start its PV while the
  previous pair finishes its normalization chain.
"""
import sys

sys.path.insert(0, "/opt/trn_rl_repo")

import ml_dtypes
import numpy as np

import concourse.bass as bass
import concourse.mybir as mybir
import concourse.tile as tile
from concourse.bass_utils import run_bass_kernel_spmd

P = 128
T, D, H, HD = 4096, 768, 12, 64
NC = 8
SQ = T // NC          # 512 q rows per core
SKV = T // NC         # 512 kv rows per chunk
DC = D // P           # 6 contraction chunks
NKB = T // P          # 32 key blocks of 128
VROW = H * (HD + 1)   # 780: V' row with ones col per head
BF = mybir.dt.bfloat16
F32R = mybir.dt.float32r
F32 = mybir.dt.float32
BF_NP = ml_dtypes.bfloat16

# kb batches: (kbs, WS, N, SLOT). Window start WS = 16*kbs[0] (16-row
# granularity: key block kb only needs q cols >= 16*kb, and the causal
# boundary of kb then falls in cols [16*(kb-kbs[0]), ...+128) of the
# window, i.e. inside the first 128 cols -- exactly what masks[r] with
# r = kb - kbs[0] encodes). Matmul PSUM outputs must not cross a
# 512-col bank boundary; slots stride 512 while N > 256.
_BATCHES = []
_kb = 0
for _g in (2, 2, 2, 2, 2, 2, 2, 2, 4, 4, 8):
    _ws = 16 * _kb
    _n = SQ - _ws
    _slot = 512 if _n > 256 else (256 if _n > 128 else 128)
    _BATCHES.append((list(range(_kb, _kb + _g)), _ws, _n, _slot))
    _kb += _g


def q_rows(c):
    t = np.arange(32)
    u = np.arange(16)
    return (16 * (c + 8 * t)[:, None] + u[None, :]).reshape(-1)


def make_masks(c):
    r = np.arange(8)[:, None, None]
    kap = np.arange(128)[None, :, None]
    j = np.arange(128)[None, None, :]
    valid = (128 * r + kap) <= (16 * c + 128 * (j // 16) + (j % 16))
    return valid.astype(BF_NP)


def fix_excess_waits(nc):
    """walrus rejects >1 sync wait per instruction; hoist extras onto NoOps."""
    k = 0
    for f in nc.m.functions:
        for bb in f.blocks:
            insts = bb.instructions
            i = 0
            while i < len(insts):
                ins = insts[i]
                si = getattr(ins, "sync_info", None)
                if si is not None and len(si.on_wait) > 1:
                    for w in si.on_wait[:-1]:
                        nop = mybir.InstNoOp(name=f"W-hoist-{k}", ins=[], outs=[])
                        k += 1
                        nop.engine = ins.engine
                        nop.sync_info = mybir.SyncInfo(on_wait=[w], on_update=[])
                        insts.insert(i, nop)
                        i += 1
                    ins.sync_info = mybir.SyncInfo(
                        on_wait=[si.on_wait[-1]], on_update=list(si.on_update))
                i += 1
    return k


# Gather groups, sized [1,1,2,2] head-pairs. Group j carries the K^T
# head-pair slices and V' head columns that attention pairs consume.
# The first gather carries ONLY pair 0, so it clears the wire ~40us
# earlier and attention starts while the bigger tail groups transfer.
_GRP = [(0, 1), (1, 2), (2, 4), (4, 6)]          # (hp_lo, hp_hi) per group
NG = len(_GRP)
_VW = 2 * (HD + 1)                               # V' cols per head pair
_XGS = [(hi - lo) * (SKV + (SKV // P) * _VW) for lo, hi in _GRP]


def build(fix_waits=True):
    nc = bass.Bass(num_devices=NC)
    xqt = nc.dram_tensor("xqt", [D, SQ], BF, kind="ExternalInput")
    xkv = nc.dram_tensor("xkv", [D, SKV], BF, kind="ExternalInput")
    kvstage = [nc.dram_tensor(f"kvstage{j}", [P, _XGS[j]], BF, kind="Internal")
               for j in range(NG)]
    gouts = [nc.dram_tensor(f"gout{j}", [NC * P, _XGS[j]], BF,
                            kind="Internal", addr_space="Shared")
             for j in range(NG)]
    dstage = nc.dram_tensor("dstage", [P, 2], BF, kind="Internal")
    dgout = nc.dram_tensor("dgout", [NC * P, 2], BF, kind="Internal",
                           addr_space="Shared")
    wq = nc.dram_tensor("wq", [D, D], BF, kind="ExternalInput")
    wk = nc.dram_tensor("wk", [D, D], BF, kind="ExternalInput")
    wv = nc.dram_tensor("wv", [D, D], BF, kind="ExternalInput")
    wo = nc.dram_tensor("wo", [D, D], BF, kind="ExternalInput")
    bo = nc.dram_tensor("bo", [P, D], F32, kind="ExternalInput")
    masks = nc.dram_tensor("masks", [8, P, P], BF, kind="ExternalInput")
    out = nc.dram_tensor("out", [SQ, D], F32, kind="ExternalOutput")

    EXP = mybir.ActivationFunctionType.Exp
    LN = mybir.ActivationFunctionType.Ln

    with tile.TileContext(nc) as tc:
        with (
            tc.tile_pool(name="glob", bufs=1) as glob,
            tc.tile_pool(name="kt", bufs=1) as ktp,
        ):
            # ---- tiles that live the whole kernel
            qt_z = glob.tile([P, H, SQ], BF)         # zero-padded per-head Q^T
            masks_sb = glob.tile([P, 8, P], BF)
            bo_bc = glob.tile([P, D], F32)
            wo_sb = glob.tile([P, DC, D], BF)
            ones_col = glob.tile([1, HD], BF)        # for denom broadcast
            v_all = glob.tile([P, NC, SKV // P, VROW], BF)   # V' resident

            # Tiny warm-up AllGather, fired before anything else: absorbs
            # the first-collective setup cost (~30us of CC ring/semaphore
            # init) in parallel with the input DMA phase, so gather group 0
            # clears the wire sooner.
            dum = glob.tile([P, 2], BF)
            nc.vector.memset(dum[:], 0.0)
            nc.sync.dma_start(dstage[:], dum[:])
            nc.gpsimd.collective_compute(
                "AllGather", mybir.AluOpType.bypass,
                replica_groups=[list(range(NC))],
                ins=[dstage[:]], outs=[dgout[:]])

            nc.vector.memset(qt_z.bitcast(mybir.dt.uint16), 0)
            nc.vector.memset(ones_col[:], 1.0)

            kt_c = [ktp.tile([P, DC, SKV], BF, name=f"ktc{r}") for r in range(NC)]

            # ===== phase 1: K^T and V' for the FULL sequence (first: its
            # DMAs gate PE start), then Q^T while attention's other inputs
            # stream in.
            with (
                tc.tile_pool(name="ph1b", bufs=1) as ph1b,
                tc.tile_pool(name="ps1b", bufs=2, space="PSUM") as ps1b,
            ):
                wq_sb = ph1b.tile([P, DC, D], BF)
                xq_sb = ph1b.tile([P, DC, SQ], BF)

                with (
                    tc.tile_pool(name="ph1a", bufs=1) as ph1a,
                    tc.tile_pool(name="ps1", bufs=2, space="PSUM") as ps1,
                ):
                    wk_sb = ph1a.tile([P, DC, D], BF)
                    wv_sb = ph1a.tile([P, DC, D], BF)
                    kt_own = ph1a.tile([P, DC, SKV], BF)
                    v_own = ph1a.tile([P, SKV // P, VROW], BF)
                    xtc = ph1a.tile([P, DC, SKV], BF)
                    nc.sync.dma_start(
                        wk_sb[:], wk.rearrange("(o p) d -> p o d", p=P))
                    nc.sync.dma_start(
                        wv_sb[:], wv.rearrange("(o p) d -> p o d", p=P))
                    nc.sync.dma_start(
                        xtc[:], xkv.rearrange("(o p) t -> p o t", p=P))
                    nc.sync.dma_start(
                        wq_sb[:], wq.rearrange("(o p) d -> p o d", p=P))
                    nc.sync.dma_start(
                        xq_sb[:], xqt.rearrange("(o p) t -> p o t", p=P))
                    nc.sync.dma_start(
                        masks_sb[:], masks.rearrange("r p j -> p r j"))
                    nc.sync.dma_start(bo_bc[:], bo[:])
                    nc.sync.dma_start(
                        wo_sb[:], wo.rearrange("(o p) d -> p o d", p=P))

                    v4o = v_own.rearrange("p o (h c) -> p o h c", c=HD + 1)
                    nc.vector.memset(v4o[:, :, :, HD:HD + 1], 1.0)

                    def k_cols(dc):
                        pp = ps1.tile([P, SKV], F32, tag="pp")
                        for ko in range(DC):
                            nc.tensor.matmul(
                                pp[:], wk_sb[:, ko, dc * P:(dc + 1) * P],
                                xtc[:, ko, :],
                                start=(ko == 0), stop=(ko == DC - 1))
                        nc.vector.tensor_copy(kt_own[:, dc, :], pp[:])

                    def v_pairs(lo, hi):
                        # V' cols for head pairs [lo, hi): 128*(hi-lo) wide
                        w = 128 * (hi - lo)
                        for tc4 in range(SKV // P):
                            pp = ps1.tile([P, w], F32, tag=f"ppv{w}")
                            for ko in range(DC):
                                nc.tensor.matmul(
                                    pp[:], xtc[:, ko, tc4 * P:(tc4 + 1) * P],
                                    wv_sb[:, ko, 128 * lo:128 * hi],
                                    start=(ko == 0), stop=(ko == DC - 1))
                            nc.vector.tensor_copy(
                                v4o[:, tc4, 2 * lo:2 * hi, 0:HD],
                                pp.rearrange("p (h c) -> p h c", c=HD))

                    def stage_and_gather(j):
                        lo, hi = _GRP[j]
                        kw = (hi - lo) * SKV
                        nc.sync.dma_start(
                            kvstage[j][:, 0:kw].rearrange(
                                "p (o c) -> p o c", c=SKV),
                            kt_own[:, lo:hi, :])
                        nc.sync.dma_start(
                            kvstage[j][:, kw:_XGS[j]].rearrange(
                                "p (o c) -> p o c", c=_VW * (hi - lo)),
                            v_own[:, :, lo * _VW:hi * _VW])
                        nc.gpsimd.collective_compute(
                            "AllGather", mybir.AluOpType.bypass,
                            replica_groups=[list(range(NC))],
                            ins=[kvstage[j][:]], outs=[gouts[j][:]])

                    # own-chunk projection ordered so each gather group
                    # launches as soon as exactly its slices exist
                    for j, (lo, hi) in enumerate(_GRP):
                        for hp in range(lo, hi):
                            k_cols(hp)
                        v_pairs(lo, hi)
                        stage_and_gather(j)
                    for j, (lo, hi) in enumerate(_GRP):
                        kw = (hi - lo) * SKV
                        for r in range(NC):
                            nc.sync.dma_start(
                                kt_c[r][:, lo:hi, :],
                                gouts[j][r * P:(r + 1) * P, 0:kw]
                                .rearrange("p (o c) -> p o c", c=SKV))
                            nc.sync.dma_start(
                                v_all[:, r, :, lo * _VW:hi * _VW],
                                gouts[j][r * P:(r + 1) * P, kw:_XGS[j]]
                                .rearrange("p (o c) -> p o c",
                                           c=_VW * (hi - lo)))

                # Q^T into zero-padded per-head slots
                for dc in range(DC):
                    pp = ps1b.tile([P, SQ], F32, tag="pp")
                    for ko in range(DC):
                        nc.tensor.matmul(
                            pp[:], wq_sb[:, ko, dc * P:(dc + 1) * P],
                            xq_sb[:, ko, :], start=(ko == 0), stop=(ko == DC - 1))
                    nc.vector.tensor_copy(qt_z[0:64, 2 * dc, :], pp[0:64, :])
                    nc.vector.tensor_copy(qt_z[64:128, 2 * dc + 1, :], pp[64:128, :])

            # ================= phase 2 + 3 ==================================
            with tc.tile_pool(name="mid", bufs=1) as mid:
                ctxt = mid.tile([P, DC, SQ], BF)     # ctx^T, d on partitions

                with (
                    tc.tile_pool(name="att", bufs=4) as att,
                    tc.tile_pool(name="ps_s", bufs=2, space="PSUM") as ps_s,
                    tc.tile_pool(name="ps_c", bufs=3, space="PSUM") as ps_c,
                    tc.tile_pool(name="ps_b", bufs=1, space="PSUM") as ps_b,
                ):
                    # heads processed in pairs, batch-interleaved: the PE runs
                    # head h+1's S^T while ACT/DVE exp+mask head h's batch.
                    for h0 in range(0, H, 2):
                        scope = nc.named_scope(f"attn{h0}")
                        scope.__enter__()
                        pair = (h0, h0 + 1)
                        cps = {h: ps_c.tile([P, SQ], F32, tag="ctx",
                                            name=f"cps{h}") for h in pair}
                        for kbs, ws, N, SLOT in _BATCHES:
                            W = len(kbs) * SLOT
                            for h in pair:
                                hp = h // 2
                                sps = ps_s.tile([P, 1024], F32, tag="s")
                                for i, kb in enumerate(kbs):
                                    nc.tensor.matmul(
                                        sps[:, i * SLOT:i * SLOT + N],
                                        kt_c[kb // 4][:, hp,
                                                      (kb % 4) * P:(kb % 4 + 1) * P],
                                        qt_z[:, h, ws:SQ],
                                        start=True, stop=True)
                                pt = att.tile([P, 1024], BF, tag="pt")
                                if SLOT != N:
                                    # exp only the valid cols (strided)
                                    pv3 = pt[:, :W].rearrange(
                                        "p (g s) -> p g s", s=SLOT)[:, :, 0:N]
                                    sv3 = sps[:, :W].rearrange(
                                        "p (g s) -> p g s", s=SLOT)[:, :, 0:N]
                                    nc.scalar.activation(pv3, sv3, EXP, scale=0.125)
                                else:
                                    nc.scalar.activation(
                                        pt[:, :W], sps[:, :W], EXP, scale=0.125)
                                ptv = pt[:, :W].rearrange("p (g n) -> p g n", n=SLOT)
                                nc.vector.tensor_mul(
                                    ptv[:, :, 0:P], ptv[:, :, 0:P],
                                    masks_sb[:, 0:len(kbs), :])
                                for i, kb in enumerate(kbs):
                                    nc.tensor.matmul(
                                        cps[h][0:HD + 1, ws:SQ],
                                        v_all[:, kb // 4, kb % 4,
                                              h * (HD + 1):(h + 1) * (HD + 1)],
                                        pt[:, i * SLOT:i * SLOT + N],
                                        start=(kb == 0), stop=(kb == NKB - 1),
                                        skip_group_check=True)
                        # 1/den = exp(-ln(den)) on ACT (DVE's InstReciprocal
                        # costs 3.3us on a 1-partition row)
                        recs = {}
                        for h in pair:
                            lnd = att.tile([1, SQ], F32, tag="lnd")
                            nc.scalar.activation(
                                lnd[:], cps[h][HD:HD + 1, :], LN)
                            rec = att.tile([1, SQ], BF, tag="rec")
                            nc.scalar.activation(rec[:], lnd[:], EXP, scale=-1.0)
                            recs[h] = rec
                        for h in pair:
                            hp, hr = h // 2, (h % 2) * 64
                            bcp = ps_b.tile([HD, SQ], F32, tag="bc")
                            nc.tensor.matmul(
                                bcp[:], ones_col[:], recs[h][:],
                                start=True, stop=True)
                            bcs = att.tile([HD, SQ], BF, tag="bcs")
                            nc.vector.tensor_copy(bcs[:], bcp[:])
                            nc.vector.tensor_mul(
                                ctxt[hr:hr + 64, hp, :], cps[h][0:64, :], bcs[:])
                        scope.__exit__(None, None, None)

                # ---- output projection
                with (
                    tc.tile_pool(name="ph3", bufs=1) as ph3,
                    tc.tile_pool(name="ps3", bufs=2, space="PSUM") as ps3,
                ):
                    o_sb = ph3.tile([P, SQ // P, D], F32)
                    outv = out.rearrange("(o p) d -> p o d", p=P)
                    for tc4 in range(SQ // P):
                        for nh in range(2):
                            op = ps3.tile([P, 384], F32, tag="op")
                            for dc in range(DC):
                                nc.tensor.matmul(
                                    op[:], ctxt[:, dc, tc4 * P:(tc4 + 1) * P],
                                    wo_sb[:, dc, nh * 384:(nh + 1) * 384],
                                    start=(dc == 0), stop=(dc == DC - 1))
                            nc.vector.tensor_add(
                                o_sb[:, tc4, nh * 384:(nh + 1) * 384], op[:],
                                bo_bc[:, nh * 384:(nh + 1) * 384])
                        # stream each 128-row block out as soon as it's done
                        nc.sync.dma_start(
                            outv[:, tc4:tc4 + 1, :], o_sb[:, tc4:tc4 + 1, :])

    if fix_waits:
        fix_excess_waits(nc)
    return nc


_NC_CACHE = None


def _get_nc():
    global _NC_CACHE
    if _NC_CACHE is None:
        _NC_CACHE = build()
    return _NC_CACHE


def _in_maps(inputs):
    x = np.asarray(inputs["x"], dtype=np.float32)
    Wq = np.asarray(inputs["Wq"], dtype=np.float32).astype(BF_NP)
    Wk = np.asarray(inputs["Wk"], dtype=np.float32).astype(BF_NP)
    Wv = np.asarray(inputs["Wv"], dtype=np.float32).astype(BF_NP)
    Wo = np.asarray(inputs["Wo"], dtype=np.float32).astype(BF_NP)
    bo_v = np.ascontiguousarray(
        np.broadcast_to(np.asarray(inputs["bo"], dtype=np.float32).reshape(1, D),
                        (P, D)))
    xf = x.reshape(T, D)
    maps = []
    for c in range(NC):
        rows = q_rows(c)
        maps.append({
            "xqt": np.ascontiguousarray(xf[rows].T).astype(BF_NP),
            "xkv": np.ascontiguousarray(
                xf[c * SKV:(c + 1) * SKV].T).astype(BF_NP),
            "wq": Wq, "wk": Wk, "wv": Wv, "wo": Wo, "bo": bo_v,
            "masks": make_masks(c),
        })
    return maps


def _run(inputs, trace=False):
    nc_prog = _get_nc()
    res = run_bass_kernel_spmd(
        nc_prog, _in_maps(inputs), core_ids=list(range(NC)), trace=trace)
    full = np.empty((T, D), dtype=np.float32)
    for c in range(NC):
        full[q_rows(c)] = res.results[c]["out"]
    return full.reshape(1, T, D), res


def kernel(**inputs) -> np.ndarray:
    out, _ = _run(inputs, trace=False)
    return out


# revision 47
# speedup vs baseline: 1.0481x; 1.0481x over previous
"""Causal MHA (B=1, T=4096, D=768, H=12) on 8 TRN2 NeuronCores.

Strategy
--------
- Sequence-parallel over T with row-interleaved q-assignment so every core
  runs the *same* program on identically-shaped causal work:
  core c owns q rows {16*(c+8*t)+u}, i.e. 16-row miniblocks strided by 8.
- K/V projection is sharded: each core projects only its own 512-key
  chunk, stages K^T+V' to DRAM, and three pipelined AllGathers (split by
  head-pair group) broadcast them. Gather group 0 (head pairs 0-1)
  launches after ~1/3 of the own-chunk projection, so attention pair 0
  starts while groups 1-2 are still on the wire. Q projection fills the
  remaining gather latency.
- All matmul inputs in bfloat16 (host-side cast): full PE rate at every
  moving-dim size (fp32r drops to 1/4 rate below 256), half the DMA
  bytes, and double DVE throughput on copies/masking. PSUM stays fp32.
- K^T and V' (with the ones-column that folds the softmax denominator
  into the PV matmul) are fully SBUF-resident in bf16.
- Score batches use 16-row-granular causal q-windows (window start
  16*kbs[0]); the per-kb masks encode the boundary with r = kb - kbs[0].
  exp() reads only valid score columns (strided APs) -- the attention
  phase is ACT(exp)-throughput-bound, so every wasted exp column counts.
- Softmax denominator: 1/den = exp(-ln(den)) on the (otherwise idle at
  pair tails) ACT engine, broadcast across the 64 head dims with a
  1-partition PE outer product.
- PSUM: 2 score bufs (2 banks each) + 3 ctx bufs + 1 broadcast buf
  = 8 banks; 3 ctx bufs let the next head pair start its PV while the
  previous pair finishes its normalization chain.
"""
import sys

sys.path.insert(0, "/opt/trn_rl_repo")

import ml_dtypes
import numpy as np

import concourse.bass as bass
import concourse.mybir as mybir
import concourse.tile as tile
from concourse.bass_utils import run_bass_kernel_spmd

P = 128
T, D, H, HD = 4096, 768, 12, 64
NC = 8
SQ = T // NC          # 512 q rows per core
SKV = T // NC         # 512 kv rows per chunk
DC = D // P           # 6 contraction chunks
NKB = T // P          # 32 key blocks of 128
VROW = H * (HD + 1)   # 780: V' row with ones col per head
BF = mybir.dt.bfloat16
F32R = mybir.dt.float32r
F32 = mybir.dt.float32
BF_NP = ml_dtypes.bfloat16

# kb batches: (kbs, WS, N, SLOT). Window start WS = 16*kbs[0] (16-row
# granularity: key block kb only needs q cols >= 16*kb, and the causal
# boundary of kb then falls in cols [16*(kb-kbs[0]), ...+128) of the
# window, i.e. inside the first 128 cols -- exactly what masks[r] with
# r = kb - kbs[0] encodes). Matmul PSUM outputs must not cross a
# 512-col bank boundary; slots stride 512 while N > 256.
_BATCHES = []
_kb = 0
for _g in (2, 2, 2, 2, 2, 2, 2, 2, 4, 4, 8):
    _ws = 16 * _kb
    _n = SQ - _ws
    _slot = 512 if _n > 256 else (256 if _n > 128 else 128)
    _BATCHES.append((list(range(_kb, _kb + _g)), _ws, _n, _slot))
    _kb += _g


def q_rows(c):
    t = np.arange(32)
    u = np.arange(16)
    return (16 * (c + 8 * t)[:, None] + u[None, :]).reshape(-1)


def make_masks(c):
    r = np.arange(8)[:, None, None]
    kap = np.arange(128)[None, :, None]
    j = np.arange(128)[None, None, :]
    valid = (128 * r + kap) <= (16 * c + 128 * (j // 16) + (j % 16))
    return valid.astype(BF_NP)


def fix_excess_waits(nc):
    """walrus rejects >1 sync wait per instruction; hoist extras onto NoOps."""
    k = 0
    for f in nc.m.functions:
        for bb in f.blocks:
            insts = bb.instructions
            i = 0
            while i < len(insts):
                ins = insts[i]
                si = getattr(ins, "sync_info", None)
                if si is not None and len(si.on_wait) > 1:
                    for w in si.on_wait[:-1]:
                        nop = mybir.InstNoOp(name=f"W-hoist-{k}", ins=[], outs=[])
                        k += 1
                        nop.engine = ins.engine
                        nop.sync_info = mybir.SyncInfo(on_wait=[w], on_update=[])
                        insts.insert(i, nop)
                        i += 1
                    ins.sync_info = mybir.SyncInfo(
                        on_wait=[si.on_wait[-1]], on_update=list(si.on_update))
                i += 1
    return k


# Gather groups, sized [1,1,2,2] head-pairs. Group j carries the K^T
# head-pair slices and V' head columns that attention pairs consume.
# The first gather carries ONLY pair 0, so it clears the wire ~40us
# earlier and attention starts while the bigger tail groups transfer.
_GRP = [(0, 1), (1, 2), (2, 4), (4, 6)]          # (hp_lo, hp_hi) per group
NG = len(_GRP)
_VW = 2 * (HD + 1)                               # V' cols per head pair
_XGS = [(hi - lo) * (SKV + (SKV // P) * _VW) for lo, hi in _GRP]


def build(fix_waits=True):
    nc = bass.Bass(num_devices=NC)
    xqt = nc.dram_tensor("xqt", [D, SQ], BF, kind="ExternalInput")
    xkv = nc.dram_tensor("xkv", [D, SKV], BF, kind="ExternalInput")
    kvstage = [nc.dram_tensor(f"kvstage{j}", [P, _XGS[j]], BF, kind="Internal")
               for j in range(NG)]
    gouts = [nc.dram_tensor(f"gout{j}", [NC * P, _XGS[j]], BF,
                            kind="Internal", addr_space="Shared")
             for j in range(NG)]
    wq = nc.dram_tensor("wq", [D, D], BF, kind="ExternalInput")
    wk = nc.dram_tensor("wk", [D, D], BF, kind="ExternalInput")
    wv = nc.dram_tensor("wv", [D, D], BF, kind="ExternalInput")
    wo = nc.dram_tensor("wo", [D, D], BF, kind="ExternalInput")
    bo = nc.dram_tensor("bo", [P, D], F32, kind="ExternalInput")
    masks = nc.dram_tensor("masks", [8, P, P], BF, kind="ExternalInput")
    out = nc.dram_tensor("out", [SQ, D], F32, kind="ExternalOutput")

    EXP = mybir.ActivationFunctionType.Exp
    LN = mybir.ActivationFunctionType.Ln

    with tile.TileContext(nc) as tc:
        with (
            tc.tile_pool(name="glob", bufs=1) as glob,
            tc.tile_pool(name="kt", bufs=1) as ktp,
        ):
            # ---- tiles that live the whole kernel
            qt_z = glob.tile([P, H, SQ], BF)         # zero-padded per-head Q^T
            masks_sb = glob.tile([P, 8, P], BF)
            bo_bc = glob.tile([P, D], F32)
            wo_sb = glob.tile([P, DC, D], BF)
            ones_col = glob.tile([1, HD], BF)        # for denom broadcast
            v_all = glob.tile([P, NC, SKV // P, VROW], BF)   # V' resident

            nc.vector.memset(qt_z.bitcast(mybir.dt.uint16), 0)
            nc.vector.memset(ones_col[:], 1.0)

            kt_c = [ktp.tile([P, DC, SKV], BF, name=f"ktc{r}") for r in range(NC)]

            # ===== phase 1: K^T and V' for the FULL sequence (first: its
            # DMAs gate PE start), then Q^T while attention's other inputs
            # stream in.
            with (
                tc.tile_pool(name="ph1b", bufs=1) as ph1b,
                tc.tile_pool(name="ps1b", bufs=2, space="PSUM") as ps1b,
            ):
                wq_sb = ph1b.tile([P, DC, D], BF)
                xq_sb = ph1b.tile([P, DC, SQ], BF)

                with (
                    tc.tile_pool(name="ph1a", bufs=1) as ph1a,
                    tc.tile_pool(name="ps1", bufs=2, space="PSUM") as ps1,
                ):
                    wk_sb = ph1a.tile([P, DC, D], BF)
                    wv_sb = ph1a.tile([P, DC, D], BF)
                    kt_own = ph1a.tile([P, DC, SKV], BF)
                    v_own = ph1a.tile([P, SKV // P, VROW], BF)
                    xtc = ph1a.tile([P, DC, SKV], BF)
                    nc.sync.dma_start(
                        wk_sb[:], wk.rearrange("(o p) d -> p o d", p=P))
                    nc.sync.dma_start(
                        xtc[:], xkv.rearrange("(o p) t -> p o t", p=P))
                    nc.sync.dma_start(
                        wv_sb[:], wv.rearrange("(o p) d -> p o d", p=P))
                    nc.sync.dma_start(
                        wq_sb[:], wq.rearrange("(o p) d -> p o d", p=P))
                    nc.sync.dma_start(
                        xq_sb[:], xqt.rearrange("(o p) t -> p o t", p=P))
                    nc.sync.dma_start(
                        masks_sb[:], masks.rearrange("r p j -> p r j"))
                    nc.sync.dma_start(bo_bc[:], bo[:])
                    nc.sync.dma_start(
                        wo_sb[:], wo.rearrange("(o p) d -> p o d", p=P))

                    v4o = v_own.rearrange("p o (h c) -> p o h c", c=HD + 1)
                    nc.vector.memset(v4o[:, :, :, HD:HD + 1], 1.0)

                    def k_cols(dc):
                        pp = ps1.tile([P, SKV], F32, tag="pp")
                        for ko in range(DC):
                            nc.tensor.matmul(
                                pp[:], wk_sb[:, ko, dc * P:(dc + 1) * P],
                                xtc[:, ko, :],
                                start=(ko == 0), stop=(ko == DC - 1))
                        nc.vector.tensor_copy(kt_own[:, dc, :], pp[:])

                    def v_pairs(lo, hi):
                        # V' cols for head pairs [lo, hi): 128*(hi-lo) wide
                        w = 128 * (hi - lo)
                        for tc4 in range(SKV // P):
                            pp = ps1.tile([P, w], F32, tag=f"ppv{w}")
                            for ko in range(DC):
                                nc.tensor.matmul(
                                    pp[:], xtc[:, ko, tc4 * P:(tc4 + 1) * P],
                                    wv_sb[:, ko, 128 * lo:128 * hi],
                                    start=(ko == 0), stop=(ko == DC - 1))
                            nc.vector.tensor_copy(
                                v4o[:, tc4, 2 * lo:2 * hi, 0:HD],
                                pp.rearrange("p (h c) -> p h c", c=HD))

                    def stage_and_gather(j):
                        lo, hi = _GRP[j]
                        kw = (hi - lo) * SKV
                        nc.sync.dma_start(
                            kvstage[j][:, 0:kw].rearrange(
                                "p (o c) -> p o c", c=SKV),
                            kt_own[:, lo:hi, :])
                        nc.sync.dma_start(
                            kvstage[j][:, kw:_XGS[j]].rearrange(
                                "p (o c) -> p o c", c=_VW * (hi - lo)),
                            v_own[:, :, lo * _VW:hi * _VW])
                        nc.gpsimd.collective_compute(
                            "AllGather", mybir.AluOpType.bypass,
                            replica_groups=[list(range(NC))],
                            ins=[kvstage[j][:]], outs=[gouts[j][:]])

                    # own-chunk projection ordered so each gather group
                    # launches as soon as exactly its slices exist. K cols
                    # 0-1 run first: they only need wk+xkv (wv lands during),
                    # so gather group 0 enters the wire a few us earlier.
                    k_cols(0), k_cols(1)
                    v_pairs(0, 1)
                    stage_and_gather(0)
                    v_pairs(1, 2)
                    stage_and_gather(1)
                    k_cols(2), k_cols(3)
                    v_pairs(2, 4)
                    stage_and_gather(2)
                    k_cols(4), k_cols(5)
                    v_pairs(4, 6)
                    stage_and_gather(3)
                    for j, (lo, hi) in enumerate(_GRP):
                        kw = (hi - lo) * SKV
                        for r in range(NC):
                            nc.sync.dma_start(
                                kt_c[r][:, lo:hi, :],
                                gouts[j][r * P:(r + 1) * P, 0:kw]
                                .rearrange("p (o c) -> p o c", c=SKV))
                            nc.sync.dma_start(
                                v_all[:, r, :, lo * _VW:hi * _VW],
                                gouts[j][r * P:(r + 1) * P, kw:_XGS[j]]
                                .rearrange("p (o c) -> p o c",
                                           c=_VW * (hi - lo)))

                # Q^T into zero-padded per-head slots
                for dc in range(DC):
                    pp = ps1b.tile([P, SQ], F32, tag="pp")
                    for ko in range(DC):
                        nc.tensor.matmul(
                            pp[:], wq_sb[:, ko, dc * P:(dc + 1) * P],
                            xq_sb[:, ko, :], start=(ko == 0), stop=(ko == DC - 1))
                    nc.vector.tensor_copy(qt_z[0:64, 2 * dc, :], pp[0:64, :])
                    nc.vector.tensor_copy(qt_z[64:128, 2 * dc + 1, :], pp[64:128, :])

            # ================= phase 2 + 3 ==================================
            with tc.tile_pool(name="mid", bufs=1) as mid:
                ctxt = mid.tile([P, DC, SQ], BF)     # ctx^T, d on partitions

                with (
                    tc.tile_pool(name="att", bufs=4) as att,
                    tc.tile_pool(name="ps_s", bufs=2, space="PSUM") as ps_s,
                    tc.tile_pool(name="ps_c", bufs=3, space="PSUM") as ps_c,
                    tc.tile_pool(name="ps_b", bufs=1, space="PSUM") as ps_b,
                ):
                    # heads processed in pairs, batch-interleaved: the PE runs
                    # head h+1's S^T while ACT/DVE exp+mask head h's batch.
                    for h0 in range(0, H, 2):
                        scope = nc.named_scope(f"attn{h0}")
                        scope.__enter__()
                        pair = (h0, h0 + 1)
                        cps = {h: ps_c.tile([P, SQ], F32, tag="ctx",
                                            name=f"cps{h}") for h in pair}
                        for kbs, ws, N, SLOT in _BATCHES:
                            W = len(kbs) * SLOT
                            for h in pair:
                                hp = h // 2
                                sps = ps_s.tile([P, 1024], F32, tag="s")
                                for i, kb in enumerate(kbs):
                                    nc.tensor.matmul(
                                        sps[:, i * SLOT:i * SLOT + N],
                                        kt_c[kb // 4][:, hp,
                                                      (kb % 4) * P:(kb % 4 + 1) * P],
                                        qt_z[:, h, ws:SQ],
                                        start=True, stop=True)
                                pt = att.tile([P, 1024], BF, tag="pt")
                                if SLOT != N:
                                    # exp only the valid cols (strided)
                                    pv3 = pt[:, :W].rearrange(
                                        "p (g s) -> p g s", s=SLOT)[:, :, 0:N]
                                    sv3 = sps[:, :W].rearrange(
                                        "p (g s) -> p g s", s=SLOT)[:, :, 0:N]
                                    nc.scalar.activation(pv3, sv3, EXP, scale=0.125)
                                else:
                                    nc.scalar.activation(
                                        pt[:, :W], sps[:, :W], EXP, scale=0.125)
                                ptv = pt[:, :W].rearrange("p (g n) -> p g n", n=SLOT)
                                nc.vector.tensor_mul(
                                    ptv[:, :, 0:P], ptv[:, :, 0:P],
                                    masks_sb[:, 0:len(kbs), :])
                                for i, kb in enumerate(kbs):
                                    nc.tensor.matmul(
                                        cps[h][0:HD + 1, ws:SQ],
                                        v_all[:, kb // 4, kb % 4,
                                              h * (HD + 1):(h + 1) * (HD + 1)],
                                        pt[:, i * SLOT:i * SLOT + N],
                                        start=(kb == 0), stop=(kb == NKB - 1),
                                        skip_group_check=True)
                        # 1/den = exp(-ln(den)) on ACT (DVE's InstReciprocal
                        # costs 3.3us on a 1-partition row)
                        recs = {}
                        for h in pair:
                            lnd = att.tile([1, SQ], F32, tag="lnd")
                            nc.scalar.activation(
                                lnd[:], cps[h][HD:HD + 1, :], LN)
                            rec = att.tile([1, SQ], BF, tag="rec")
                            nc.scalar.activation(rec[:], lnd[:], EXP, scale=-1.0)
                            recs[h] = rec
                        for h in pair:
                            hp, hr = h // 2, (h % 2) * 64
                            bcp = ps_b.tile([HD, SQ], F32, tag="bc")
                            nc.tensor.matmul(
                                bcp[:], ones_col[:], recs[h][:],
                                start=True, stop=True)
                            bcs = att.tile([HD, SQ], BF, tag="bcs")
                            nc.vector.tensor_copy(bcs[:], bcp[:])
                            nc.vector.tensor_mul(
                                ctxt[hr:hr + 64, hp, :], cps[h][0:64, :], bcs[:])
                        scope.__exit__(None, None, None)

                # ---- output projection
                with (
                    tc.tile_pool(name="ph3", bufs=1) as ph3,
                    tc.tile_pool(name="ps3", bufs=2, space="PSUM") as ps3,
                ):
                    o_sb = ph3.tile([P, SQ // P, D], F32)
                    outv = out.rearrange("(o p) d -> p o d", p=P)
                    for tc4 in range(SQ // P):
                        for nh in range(2):
                            op = ps3.tile([P, 384], F32, tag="op")
                            for dc in range(DC):
                                nc.tensor.matmul(
                                    op[:], ctxt[:, dc, tc4 * P:(tc4 + 1) * P],
                                    wo_sb[:, dc, nh * 384:(nh + 1) * 384],
                                    start=(dc == 0), stop=(dc == DC - 1))
                            nc.vector.tensor_add(
                                o_sb[:, tc4, nh * 384:(nh + 1) * 384], op[:],
                                bo_bc[:, nh * 384:(nh + 1) * 384])
                        # stream each 128-row block out as soon as it's done
                        nc.sync.dma_start(
                            outv[:, tc4:tc4 + 1, :], o_sb[:, tc4:tc4 + 1, :])

    if fix_waits:
        fix_excess_waits(nc)
    return nc


_NC_CACHE = None


def _get_nc():
    global _NC_CACHE
    if _NC_CACHE is None:
        _NC_CACHE = build()
    return _NC_CACHE


def _in_maps(inputs):
    x = np.asarray(inputs["x"], dtype=np.float32)
    Wq = np.asarray(inputs["Wq"], dtype=np.float32).astype(BF_NP)
    Wk = np.asarray(inputs["Wk"], dtype=np.float32).astype(BF_NP)
    Wv = np.asarray(inputs["Wv"], dtype=np.float32).astype(BF_NP)
    Wo = np.asarray(inputs["Wo"], dtype=np.float32).astype(BF_NP)
    bo_v = np.ascontiguousarray(
        np.broadcast_to(np.asarray(inputs["bo"], dtype=np.float32).reshape(1, D),
                        (P, D)))
    xf = x.reshape(T, D)
    maps = []
    for c in range(NC):
        rows = q_rows(c)
        maps.append({
            "xqt": np.ascontiguousarray(xf[rows].T).astype(BF_NP),
            "xkv": np.ascontiguousarray(
                xf[c * SKV:(c + 1) * SKV].T).astype(BF_NP),
            "wq": Wq, "wk": Wk, "wv": Wv, "wo": Wo, "bo": bo_v,
            "masks": make_masks(c),
        })
    return maps


def _run(inputs, trace=False):
    nc_prog = _get_nc()
    res = run_bass_kernel_spmd(
        nc_prog, _in_maps(inputs), core_ids=list(range(NC)), trace=trace)
    full = np.empty((T, D), dtype=np.float32)
    for c in range(NC):
        full[q_rows(c)] = res.results[c]["out"]
    return full.reshape(1, T, D), res


def kernel(**inputs) -> np.ndarray:
    out, _ = _run(inputs, trace=False)
    return out
